# revision 12
# baseline (speedup 1.0000x reference)
"""Trainium2 Bass kernel for nn_ConvAttention: LayerNorm -> 1x1-conv QKV ->
per-(b,h)-row attention over W -> skip connection.

Sharding: data-parallel over batch B=8 across 8 NeuronCores. Each core
processes 64 (h) slabs of [W=256, C=256].

Numerics strategy: every matmul runs in f32r (fp32 bits streamed through the
PE), which at moving-dim >= 256 runs at full bf16 rate (1 cycle/row) with
~2^-13.5 per-product precision -- measured on hardware, and identical to what
the 4x-slower fp32 mode produces. That precision gives score errors ~1e-2
absmax (softmax-weight rel err <1%), final output rel err ~1e-4 vs the 2e-2
gate -- so no hi/lo operand splits are needed anywhere, which is what makes
this kernel ~3x leaner on the PE than a split-based fp32 scheme.

Softmax max-subtraction is replaced by a constant shift (exact in real
arithmetic; scores are bounded well inside fp32 exp range), computing only
TRANSPOSED scores s^T = k @ q^T and exponentiating those directly -- no PE
transpose of the softmax weights. Z comes free from a ones column appended to
the V operand of the output matmul.

The slab loop is software-pipelined ~8 deep so that every cross-engine
dependency is produced at least one iteration before it is consumed; per-slab
engine busy is balanced at ~1.8-1.9us across PE/ACT/DVE/GpSimd.
"""

import os
import sys

for _p in ("/opt/trn_rl_repo", "/root/.axon_site/_ro/trn_rl_repo"):
    if _p not in sys.path:
        sys.path.insert(0, _p)

import numpy as np

import concourse.tile as tile
from concourse import bacc, mybir
from concourse.bass_utils import run_bass_kernel_spmd
from concourse.masks import make_identity

F32 = mybir.dt.float32
F32R = mybir.dt.float32r
BF16 = mybir.dt.bfloat16
AF = mybir.ActivationFunctionType
ALU = mybir.AluOpType

B, H, W, C = 8, 64, 256, 256
F2 = 2 * C
NS = H  # slabs per core (batch-sharded over 8 cores)
EPS = 1e-3  # Keras LayerNormalization default
SHIFT = 32.0  # constant softmax shift (replaces per-row max subtraction)

_NC_CACHE: dict = {}


def _install_act_root():
    """Reorder act_info.json so natural_log_exp_and_others is the first set:
    bass' first-match table chooser then resolves both Ln and Exp to that one
    set instead of alternating exp_and_others / natural_log every slab
    (129 table loads x ~1.3us). Both bass (get_activation_tables) and walrus
    (--act-root-json via BASS_ACT_ROOT_JSON_PATH) must read the same file so
    the pre-placed set ids stay in range."""
    if os.environ.get("BASS_ACT_ROOT_JSON_PATH"):
        return
    try:
        import json
        import tempfile

        import neuronxcc.driver.jobs.support.FindActInfo as FAI
        from neuronxcc.driver.Job import Job

        src = FAI.findActInfoFile(Job.getPackageDir(), "gen3")
        srcdir = os.path.dirname(src)
        d = json.load(open(src))
        sets = d["act_func_sets"]
        first = [s for s in sets if s["name"] == "natural_log_exp_and_others"]
        if not first:
            return
        rest = [s for s in sets if s["name"] != "natural_log_exp_and_others"]
        d["act_func_sets"] = first + rest
        td = tempfile.mkdtemp(prefix="act_root_")
        for fn in os.listdir(srcdir):
            sp = os.path.join(srcdir, fn)
            if os.path.isfile(sp) and fn != os.path.basename(src):
                os.symlink(sp, os.path.join(td, fn))
        out = os.path.join(td, os.path.basename(src))
        with open(out, "w") as f:
            json.dump(d, f)
        os.environ["BASS_ACT_ROOT_JSON_PATH"] = out
        # bass side reads via findActInfoFile; point it at the same file
        _orig = FAI.findActInfoFile
        FAI.findActInfoFile = lambda *a, **k: out
        import concourse.hw_specs as hw_specs

        hw_specs.get_activation_tables.cache_clear()
    except Exception as e:  # noqa: BLE001
        print(f"act root override failed (table thrash will persist): {e}")


def _build(with_bias: bool):
    _install_act_root()
    nc = bacc.Bacc("TRN2", target_bir_lowering=False, debug=False, num_devices=8)
    x_d = nc.dram_tensor("x", [NS, W, C], F32, kind="ExternalInput").ap()
    wqk_d = nc.dram_tensor("wqk", [2, 128, 256], F32R, kind="ExternalInput").ap()
    wv_d = nc.dram_tensor("wv", [2, 128, 256], F32R, kind="ExternalInput").ap()
    bqk_d = bv_d = None
    if with_bias:
        bqk_d = nc.dram_tensor("bqk", [2, 128], F32, kind="ExternalInput").ap()
        bv_d = nc.dram_tensor("bv", [256], F32, kind="ExternalInput").ap()
    out_d = nc.dram_tensor("out", [NS, W, C], F32, kind="ExternalOutput").ap()

    # per-slab views: [p=128, t(w-chunk)=2, c=256]
    x_r = x_d.rearrange("s (t p) c -> s p t c", p=128)
    out_r = out_d.rearrange("s (t p) c -> s p t c", p=128)

    with tile.TileContext(nc) as tc:
        _emit(nc, tc, x_r, out_r, wqk_d, wv_d, bqk_d, bv_d)
    nc.compile()
    return nc


def _emit(nc, tc, x_r, out_r, wqk_d, wv_d, bqk_d, bv_d):
    from contextlib import ExitStack

    def rr(ap):
        return ap.bitcast(F32R)

    with ExitStack() as ctx:
        ec = ctx.enter_context
        consts = ec(tc.tile_pool(name="consts", bufs=1))
        xpool = ec(tc.tile_pool(name="xp", bufs=12))
        stpool = ec(tc.tile_pool(name="stp", bufs=3))
        mvpool = ec(tc.tile_pool(name="mvp", bufs=5))
        rspool = ec(tc.tile_pool(name="rsp", bufs=4))
        xnpool = ec(tc.tile_pool(name="xnp", bufs=4))
        xtpool = ec(tc.tile_pool(name="xtp", bufs=4))
        qkpool = ec(tc.tile_pool(name="qkp", bufs=4))
        epool = ec(tc.tile_pool(name="ep", bufs=4))
        vpool = ec(tc.tile_pool(name="vp", bufs=5))
        opool = ec(tc.tile_pool(name="op", bufs=4))
        rzpool = ec(tc.tile_pool(name="rzp", bufs=3))
        ps_xnT = ec(tc.tile_pool(name="ps_xnT", bufs=2, space="PSUM"))
        ps_qk = ec(tc.tile_pool(name="ps_qk", bufs=1, space="PSUM"))
        ps_sT = ec(tc.tile_pool(name="ps_sT", bufs=1, space="PSUM"))
        ps_v = ec(tc.tile_pool(name="ps_v", bufs=2, space="PSUM"))
        ps_y = ec(tc.tile_pool(name="ps_y", bufs=1, space="PSUM"))

        ident_f = consts.tile([128, 128], F32)
        make_identity(nc, ident_f)
        ident = consts.tile([128, 128], F32R)
        nc.vector.tensor_copy(ident, ident_f)
        negshift = consts.tile([128, 1], F32)
        nc.vector.memset(negshift, -SHIFT)
        eps_t = consts.tile([128, 1], F32)
        nc.vector.memset(eps_t, EPS)

        ones_r = consts.tile([128, 2, 2], BF16)
        nc.vector.memset(ones_r, 1.0)

        wqk = consts.tile([128, 2, 256], F32R)
        nc.sync.dma_start(wqk, wqk_d.rearrange("t p f -> p t f"))
        wv = consts.tile([128, 2, 256], F32R)
        nc.sync.dma_start(wv, wv_d.rearrange("t p f -> p t f"))

        if bqk_d is not None:
            import concourse.bass as bass

            bqk_sb = consts.tile([128, 2], F32)
            nc.sync.dma_start(bqk_sb, bqk_d.rearrange("t p -> p t"))
            bvf = consts.tile([128, 2, 256], F32)
            bv_b = bass.AP(tensor=bv_d.tensor, offset=bv_d.offset,
                           ap=[[0, 128], [0, 2], [1, 256]])
            nc.sync.dma_start(bvf, bv_b)

        # pipeline state, keyed by slab index
        P: dict = {}

        def a1_dma_in(s):
            x_sb = xpool.tile([128, 2, 256], F32)
            nc.sync.dma_start(x_sb, x_r[s])
            P[s] = {"x": x_sb}

        def a2_stats(s):
            p = P[s]
            st = stpool.tile([128, 2, 6], F32)
            mv = mvpool.tile([128, 2, 2], F32)
            for t in (0, 1):
                nc.vector.bn_stats(st[:, t, :], p["x"][:, t, :])
                nc.vector.bn_aggr(mv[:, t, :], st[:, t, :])
            p["mv"] = mv

        def a2b_rs(s):
            p = P[s]
            # rs = rsqrt(var + eps) = exp(-0.5 * ln(var + eps)); ln+exp live
            # in one ACT table set (see _install_act_root)
            lnv = rspool.tile([128, 2, 1], F32)
            nc.scalar.activation(out=lnv, in_=p["mv"][:, :, 1:2], func=AF.Ln,
                                 bias=eps_t, scale=1.0)
            rs = rspool.tile([128, 2, 1], F32)
            nc.scalar.activation(out=rs, in_=lnv, func=AF.Exp, scale=-0.5)
            p["rs"] = rs

        def a3_norm(s):
            p = P[s]
            xn = xnpool.tile([128, 2, 256], F32R)
            for t in (0, 1):
                nc.vector.tensor_scalar(
                    out=xn[:, t, :], in0=p["x"][:, t, :],
                    scalar1=p["mv"][:, t, 0:1], scalar2=p["rs"][:, t, :],
                    op0=ALU.subtract, op1=ALU.mult)
            p["xn"] = xn

        def a4_transpose(s):
            p = P[s]
            p_xnT = ps_xnT.tile([128, 2, 256], F32)
            for cc in (0, 1):
                for t in (0, 1):
                    nc.tensor.transpose(
                        rr(p_xnT[:, cc, t * 128:(t + 1) * 128]),
                        p["xn"][:, t, cc * 128:(cc + 1) * 128],
                        ident[:, 0:128])
            xnT = xtpool.tile([128, 2, 256], F32R)
            nc.scalar.copy(xnT, p_xnT)
            p["xnT"] = xnT

        def b_proj(s):
            p = P[s]
            xnT = p["xnT"]
            vv = vpool.tile([128, 2, 258], BF16)
            nc.gpsimd.tensor_copy(vv[:, :, 256:258], ones_r)
            p_qk = ps_qk.tile([128, 2, 256], F32)
            for blk in (0, 1):
                for cc in (0, 1):
                    nc.tensor.matmul(
                        p_qk[:, blk, :],
                        wqk[:, cc, blk * 128:(blk + 1) * 128],
                        xnT[:, cc, :],
                        start=(cc == 0), stop=(cc == 1))
            qkT = qkpool.tile([128, 2, 256], F32R)
            nc.scalar.copy(qkT, p_qk)
            if bqk_d is not None:
                for blk in (0, 1):
                    nc.vector.tensor_scalar(
                        out=qkT[:, blk, :], in0=qkT[:, blk, :],
                        scalar1=bqk_sb[:, blk:blk + 1], scalar2=None,
                        op0=ALU.add)
            p_v = ps_v.tile([128, 2, 256], F32)
            for jt in (0, 1):
                for cc in (0, 1):
                    nc.tensor.matmul(
                        p_v[:, jt, :],
                        xnT[:, cc, jt * 128:(jt + 1) * 128],
                        wv[:, cc, :],
                        start=(cc == 0), stop=(cc == 1))
            nc.scalar.copy(vv[:, 0, 0:256], p_v[:, 0, :])
            nc.vector.tensor_copy(vv[:, 1, 0:256], p_v[:, 1, :])
            if bv_d is not None:
                nc.gpsimd.tensor_tensor(out=vv[:, :, 0:256],
                                        in0=vv[:, :, 0:256], in1=bvf,
                                        op=ALU.add)
            p["qkT"] = qkT
            p["vv"] = vv

        def c_scores(s):
            p = P[s]
            qkT = p["qkT"]
            p_sT = ps_sT.tile([128, 2, 256], F32)
            for jt in (0, 1):
                nc.tensor.matmul(
                    p_sT[:, jt, :],
                    qkT[:, 1, jt * 128:(jt + 1) * 128],
                    qkT[:, 0, :],
                    start=True, stop=True)
            E = epool.tile([128, 2, 256], BF16)
            nc.scalar.activation(out=E, in_=p_sT, func=AF.Exp,
                                 bias=negshift, scale=1.0)
            p["E"] = E

        def d_y(s):
            p = P[s]
            E, vv = p["E"], p["vv"]
            p_y = ps_y.tile([128, 2, 512], F32)
            for it in (0, 1):
                for jt in (0, 1):
                    nc.tensor.matmul(
                        p_y[:, it, 0:258],
                        E[:, jt, it * 128:(it + 1) * 128],
                        vv[:, jt, :],
                        start=(jt == 0), stop=(jt == 1))
            p["p_y"] = p_y

        def dt_out(s):
            p = P.pop(s)
            rZ = rzpool.tile([128, 2, 1], F32)
            nc.vector.reciprocal_approx_fast(rZ, p["p_y"][:, :, 256:257])
            o = opool.tile([128, 2, 256], F32)
            for it in (0, 1):
                nc.vector.scalar_tensor_tensor(
                    out=o[:, it, :], in0=p["p_y"][:, it, 0:256],
                    scalar=rZ[:, it, :], in1=p["x"][:, it, :],
                    op0=ALU.mult, op1=ALU.add)
            nc.sync.dma_start(out_r[s], o)

        def valid(s):
            return 0 <= s < NS

        # stage offsets chosen so every cross-engine dependency is produced
        # at least one iteration before its consumer (see module docstring)
        for k in range(NS + 8):
            if valid(k - 8):
                dt_out(k - 8)
            if valid(k):
                a1_dma_in(k)
            if valid(k - 3):
                a3_norm(k - 3)
            if valid(k - 4):
                a4_transpose(k - 4)
            if valid(k - 1):
                a2_stats(k - 1)
            if valid(k - 2):
                a2b_rs(k - 2)
            if valid(k - 5):
                b_proj(k - 5)
            if valid(k - 6):
                c_scores(k - 6)
            if valid(k - 7):
                d_y(k - 7)


def _install_ntff_hook():
    """Register the axon NTFF profiling hook (the image's antenv lacks
    axon_hooks, so boot skipped registration). Trace-only; best-effort."""
    try:
        import types

        import antenv

        if getattr(antenv, "axon_hooks", None) is not None:
            return
        mod = types.ModuleType("antenv.axon_hooks")
        _h = [None]
        mod.set_axon_ntff_profile_hook = lambda h: _h.__setitem__(0, h)
        mod.get_axon_ntff_profile_hook = lambda: _h[0]
        sys.modules["antenv.axon_hooks"] = mod
        antenv.axon_hooks = mod
        from trn_agent_boot.trn_boot import _ntff_profile_via_ctypes

        hook = _ntff_profile_via_ctypes("/opt/axon/libaxon_pjrt.so")
        if hook is not None:
            mod.set_axon_ntff_profile_hook(hook)
    except Exception as e:  # noqa: BLE001
        print(f"ntff hook install failed (timing unavailable): {e}")


def kernel(x, ln_gamma, ln_beta, W_qkv):
    x = np.asarray(x, dtype=np.float32)
    ln_gamma = np.asarray(ln_gamma, dtype=np.float32)
    ln_beta = np.asarray(ln_beta, dtype=np.float32)
    W_qkv = np.asarray(W_qkv, dtype=np.float32)
    assert x.shape == (B, H, W, C) and W_qkv.shape == (C, F2)

    # fold gamma/beta into the projection (1x1 conv has no bias of its own)
    Wp = (ln_gamma.astype(np.float64)[:, None]
          * W_qkv.astype(np.float64)).astype(np.float32)
    bW = (ln_beta.astype(np.float64)
          @ W_qkv.astype(np.float64)).astype(np.float32)
    with_bias = bool(np.any(bW != 0.0))

    key = with_bias
    if key not in _NC_CACHE:
        _NC_CACHE[key] = _build(with_bias)
    nc = _NC_CACHE[key]

    in_maps = []
    for b in range(B):
        m = {
            "x": np.ascontiguousarray(x[b]),
            "wqk": np.ascontiguousarray(Wp[:, :256].reshape(2, 128, 256)),
            "wv": np.ascontiguousarray(Wp[:, 256:].reshape(2, 128, 256)),
        }
        if with_bias:
            m["bqk"] = np.ascontiguousarray(bW[:256].reshape(2, 128))
            m["bv"] = np.ascontiguousarray(bW[256:])
        in_maps.append(m)

    trace = os.environ.get("KERNEL_TRACE", "") == "1"
    if trace:
        _install_ntff_hook()
    res = run_bass_kernel_spmd(nc, in_maps, core_ids=list(range(B)), trace=trace)
    if trace and res.exec_time_ns is not None:
        print(f"HW exec time: {res.exec_time_ns} ns")
        if res.instructions_and_trace is not None:
            print(f"trace: {res.instructions_and_trace[1]}")
    out = np.stack([res.results[b]["out"] for b in range(B)], axis=0)
    return out.reshape(B, H, W, C).astype(np.float32, copy=False)


# revision 13
# speedup vs baseline: 1.1494x; 1.1494x over previous
"""Trainium2 Bass kernel for nn_ConvAttention: LayerNorm -> 1x1-conv QKV ->
per-(b,h)-row attention over W -> skip connection.

Sharding: data-parallel over batch B=8 across 8 NeuronCores. Each core
processes 64 (h) slabs of [W=256, C=256].

Numerics strategy: every matmul runs in f32r (fp32 bits streamed through the
PE), which at moving-dim >= 256 runs at full bf16 rate (1 cycle/row) with
~2^-13.5 per-product precision -- measured on hardware, and identical to what
the 4x-slower fp32 mode produces. That precision gives score errors ~1e-2
absmax (softmax-weight rel err <1%), final output rel err ~1e-4 vs the 2e-2
gate -- so no hi/lo operand splits are needed anywhere, which is what makes
this kernel ~3x leaner on the PE than a split-based fp32 scheme.

Softmax max-subtraction is replaced by a constant shift (exact in real
arithmetic; scores are bounded well inside fp32 exp range), computing only
TRANSPOSED scores s^T = k @ q^T and exponentiating those directly -- no PE
transpose of the softmax weights. Z comes free from a ones column appended to
the V operand of the output matmul.

The slab loop is software-pipelined ~8 deep so that every cross-engine
dependency is produced at least one iteration before it is consumed; per-slab
engine busy is balanced at ~1.8-1.9us across PE/ACT/DVE/GpSimd.
"""

import os
import sys

for _p in ("/opt/trn_rl_repo", "/root/.axon_site/_ro/trn_rl_repo"):
    if _p not in sys.path:
        sys.path.insert(0, _p)

import numpy as np

import concourse.tile as tile
from concourse import bacc, mybir
from concourse.bass_utils import run_bass_kernel_spmd
from concourse.masks import make_identity

F32 = mybir.dt.float32
F32R = mybir.dt.float32r
BF16 = mybir.dt.bfloat16
AF = mybir.ActivationFunctionType
ALU = mybir.AluOpType

B, H, W, C = 8, 64, 256, 256
F2 = 2 * C
NS = H  # slabs per core (batch-sharded over 8 cores)
EPS = 1e-3  # Keras LayerNormalization default
SHIFT = 32.0  # constant softmax shift (replaces per-row max subtraction)

_NC_CACHE: dict = {}


def _install_act_root():
    """Reorder act_info.json so natural_log_exp_and_others is the first set:
    bass' first-match table chooser then resolves both Ln and Exp to that one
    set instead of alternating exp_and_others / natural_log every slab
    (129 table loads x ~1.3us). Both bass (get_activation_tables) and walrus
    (--act-root-json via BASS_ACT_ROOT_JSON_PATH) must read the same file so
    the pre-placed set ids stay in range."""
    if os.environ.get("BASS_ACT_ROOT_JSON_PATH"):
        return
    try:
        import json
        import tempfile

        import neuronxcc.driver.jobs.support.FindActInfo as FAI
        from neuronxcc.driver.Job import Job

        src = FAI.findActInfoFile(Job.getPackageDir(), "gen3")
        srcdir = os.path.dirname(src)
        d = json.load(open(src))
        sets = d["act_func_sets"]
        first = [s for s in sets if s["name"] == "natural_log_exp_and_others"]
        if not first:
            return
        rest = [s for s in sets if s["name"] != "natural_log_exp_and_others"]
        d["act_func_sets"] = first + rest
        td = tempfile.mkdtemp(prefix="act_root_")
        for fn in os.listdir(srcdir):
            sp = os.path.join(srcdir, fn)
            if os.path.isfile(sp) and fn != os.path.basename(src):
                os.symlink(sp, os.path.join(td, fn))
        out = os.path.join(td, os.path.basename(src))
        with open(out, "w") as f:
            json.dump(d, f)
        os.environ["BASS_ACT_ROOT_JSON_PATH"] = out
        # bass side reads via findActInfoFile; point it at the same file
        _orig = FAI.findActInfoFile
        FAI.findActInfoFile = lambda *a, **k: out
        import concourse.hw_specs as hw_specs

        hw_specs.get_activation_tables.cache_clear()
    except Exception as e:  # noqa: BLE001
        print(f"act root override failed (table thrash will persist): {e}")


def _build(with_bias: bool):
    _install_act_root()
    nc = bacc.Bacc("TRN2", target_bir_lowering=False, debug=False, num_devices=8)
    x_d = nc.dram_tensor("x", [NS, W, C], F32, kind="ExternalInput").ap()
    wqk_d = nc.dram_tensor("wqk", [2, 128, 256], F32R, kind="ExternalInput").ap()
    wv_d = nc.dram_tensor("wv", [2, 128, 256], F32R, kind="ExternalInput").ap()
    bqk_d = bv_d = None
    if with_bias:
        bqk_d = nc.dram_tensor("bqk", [2, 128], F32, kind="ExternalInput").ap()
        bv_d = nc.dram_tensor("bv", [256], F32, kind="ExternalInput").ap()
    out_d = nc.dram_tensor("out", [NS, W, C], F32, kind="ExternalOutput").ap()

    # per-slab views: [p=128, t(w-chunk)=2, c=256]
    x_r = x_d.rearrange("s (t p) c -> s p t c", p=128)
    out_r = out_d.rearrange("s (t p) c -> s p t c", p=128)

    with tile.TileContext(nc) as tc:
        _emit(nc, tc, x_r, out_r, wqk_d, wv_d, bqk_d, bv_d)
    nc.compile()
    return nc


def _emit(nc, tc, x_r, out_r, wqk_d, wv_d, bqk_d, bv_d):
    from contextlib import ExitStack

    def rr(ap):
        return ap.bitcast(F32R)

    with ExitStack() as ctx:
        ec = ctx.enter_context
        consts = ec(tc.tile_pool(name="consts", bufs=1))
        xpool = ec(tc.tile_pool(name="xp", bufs=12))
        stpool = ec(tc.tile_pool(name="stp", bufs=3))
        mvpool = ec(tc.tile_pool(name="mvp", bufs=5))
        rspool = ec(tc.tile_pool(name="rsp", bufs=4))
        xnpool = ec(tc.tile_pool(name="xnp", bufs=4))
        xtpool = ec(tc.tile_pool(name="xtp", bufs=4))
        qkpool = ec(tc.tile_pool(name="qkp", bufs=4))
        epool = ec(tc.tile_pool(name="ep", bufs=4))
        vpool = ec(tc.tile_pool(name="vp", bufs=5))
        opool = ec(tc.tile_pool(name="op", bufs=4))
        rzpool = ec(tc.tile_pool(name="rzp", bufs=3))
        ps_xnT = ec(tc.tile_pool(name="ps_xnT", bufs=2, space="PSUM"))
        ps_qk = ec(tc.tile_pool(name="ps_qk", bufs=1, space="PSUM"))
        ps_sT = ec(tc.tile_pool(name="ps_sT", bufs=1, space="PSUM"))
        ps_v = ec(tc.tile_pool(name="ps_v", bufs=2, space="PSUM"))
        ps_y = ec(tc.tile_pool(name="ps_y", bufs=1, space="PSUM"))

        ident_f = consts.tile([128, 128], F32)
        make_identity(nc, ident_f)
        ident = consts.tile([128, 128], F32R)
        nc.vector.tensor_copy(ident, ident_f)
        negshift = consts.tile([128, 1], F32)
        nc.vector.memset(negshift, -SHIFT)
        eps_t = consts.tile([128, 1], F32)
        nc.vector.memset(eps_t, EPS)

        ones_r = consts.tile([128, 2, 2], BF16)
        nc.vector.memset(ones_r, 1.0)

        wqk = consts.tile([128, 2, 256], F32R)
        nc.sync.dma_start(wqk, wqk_d.rearrange("t p f -> p t f"))
        wv = consts.tile([128, 2, 256], F32R)
        nc.sync.dma_start(wv, wv_d.rearrange("t p f -> p t f"))

        if bqk_d is not None:
            import concourse.bass as bass

            bqk_sb = consts.tile([128, 2], F32)
            nc.sync.dma_start(bqk_sb, bqk_d.rearrange("t p -> p t"))
            bvf = consts.tile([128, 2, 256], F32)
            bv_b = bass.AP(tensor=bv_d.tensor, offset=bv_d.offset,
                           ap=[[0, 128], [0, 2], [1, 256]])
            nc.sync.dma_start(bvf, bv_b)

        # pipeline state, keyed by slab index
        P: dict = {}

        def a1_dma_in(s):
            x_sb = xpool.tile([128, 2, 256], F32)
            nc.sync.dma_start(x_sb, x_r[s])
            P[s] = {"x": x_sb}

        def a2_stats(s):
            p = P[s]
            st = stpool.tile([128, 2, 6], F32)
            mv = mvpool.tile([128, 2, 2], F32)
            for t in (0, 1):
                nc.vector.bn_stats(st[:, t, :], p["x"][:, t, :])
                nc.vector.bn_aggr(mv[:, t, :], st[:, t, :])
            p["mv"] = mv

        def a2b_rs(s):
            p = P[s]
            # rs = rsqrt(var + eps) = exp(-0.5 * ln(var + eps)); ln+exp live
            # in one ACT table set (see _install_act_root)
            lnv = rspool.tile([128, 2, 1], F32)
            nc.scalar.activation(out=lnv, in_=p["mv"][:, :, 1:2], func=AF.Ln,
                                 bias=eps_t, scale=1.0)
            rs = rspool.tile([128, 2, 1], F32)
            nc.scalar.activation(out=rs, in_=lnv, func=AF.Exp, scale=-0.5)
            p["rs"] = rs

        def a3_norm(s):
            p = P[s]
            xn = xnpool.tile([128, 2, 256], F32R)
            for t in (0, 1):
                nc.vector.tensor_scalar(
                    out=xn[:, t, :], in0=p["x"][:, t, :],
                    scalar1=p["mv"][:, t, 0:1], scalar2=p["rs"][:, t, :],
                    op0=ALU.subtract, op1=ALU.mult)
            p["xn"] = xn

        def a4_transpose(s):
            p = P[s]
            p_xnT = ps_xnT.tile([128, 2, 256], F32)
            for cc in (0, 1):
                for t in (0, 1):
                    nc.tensor.transpose(
                        rr(p_xnT[:, cc, t * 128:(t + 1) * 128]),
                        p["xn"][:, t, cc * 128:(cc + 1) * 128],
                        ident[:, 0:128])
            xnT = xtpool.tile([128, 2, 256], F32R)
            nc.scalar.copy(xnT, p_xnT)
            p["xnT"] = xnT

        def b_proj(s):
            p = P[s]
            xnT = p["xnT"]
            vv = vpool.tile([128, 2, 258], BF16)
            nc.gpsimd.tensor_copy(vv[:, :, 256:258], ones_r)
            p_qk = ps_qk.tile([128, 2, 256], F32)
            for blk in (0, 1):
                for cc in (0, 1):
                    nc.tensor.matmul(
                        p_qk[:, blk, :],
                        wqk[:, cc, blk * 128:(blk + 1) * 128],
                        xnT[:, cc, :],
                        start=(cc == 0), stop=(cc == 1))
            qkT = qkpool.tile([128, 2, 256], F32R)
            nc.scalar.copy(qkT, p_qk)
            if bqk_d is not None:
                for blk in (0, 1):
                    nc.vector.tensor_scalar(
                        out=qkT[:, blk, :], in0=qkT[:, blk, :],
                        scalar1=bqk_sb[:, blk:blk + 1], scalar2=None,
                        op0=ALU.add)
            p_v = ps_v.tile([128, 2, 256], F32)
            for jt in (0, 1):
                for cc in (0, 1):
                    nc.tensor.matmul(
                        p_v[:, jt, :],
                        xnT[:, cc, jt * 128:(jt + 1) * 128],
                        wv[:, cc, :],
                        start=(cc == 0), stop=(cc == 1))
            nc.scalar.copy(vv[:, 0, 0:256], p_v[:, 0, :])
            nc.vector.tensor_copy(vv[:, 1, 0:256], p_v[:, 1, :])
            if bv_d is not None:
                nc.gpsimd.tensor_tensor(out=vv[:, :, 0:256],
                                        in0=vv[:, :, 0:256], in1=bvf,
                                        op=ALU.add)
            p["qkT"] = qkT
            p["vv"] = vv

        def c_scores(s):
            p = P[s]
            qkT = p["qkT"]
            p_sT = ps_sT.tile([128, 2, 256], F32)
            for jt in (0, 1):
                nc.tensor.matmul(
                    p_sT[:, jt, :],
                    qkT[:, 1, jt * 128:(jt + 1) * 128],
                    qkT[:, 0, :],
                    start=True, stop=True)
            E = epool.tile([128, 2, 256], BF16)
            nc.scalar.activation(out=E, in_=p_sT, func=AF.Exp,
                                 bias=negshift, scale=1.0)
            p["E"] = E

        def d_y(s):
            p = P[s]
            E, vv = p["E"], p["vv"]
            p_y = ps_y.tile([128, 2, 512], F32)
            for it in (0, 1):
                for jt in (0, 1):
                    nc.tensor.matmul(
                        p_y[:, it, 0:258],
                        E[:, jt, it * 128:(it + 1) * 128],
                        vv[:, jt, :],
                        start=(jt == 0), stop=(jt == 1))
            p["p_y"] = p_y

        def dt_out(s):
            p = P.pop(s)
            rZ = rzpool.tile([128, 2, 1], F32)
            nc.vector.reciprocal_approx_fast(rZ, p["p_y"][:, :, 256:257])
            o = opool.tile([128, 2, 256], F32)
            for it in (0, 1):
                nc.vector.scalar_tensor_tensor(
                    out=o[:, it, :], in0=p["p_y"][:, it, 0:256],
                    scalar=rZ[:, it, :], in1=p["x"][:, it, :],
                    op0=ALU.mult, op1=ALU.add)
            nc.sync.dma_start(out_r[s], o)

        def valid(s):
            return 0 <= s < NS

        # stage offsets chosen so every cross-engine dependency is produced
        # at least one iteration before its consumer (see module docstring)
        for k in range(NS + 8):
            if valid(k):
                a1_dma_in(k)
            if valid(k - 3):
                a3_norm(k - 3)
            if valid(k - 4):
                a4_transpose(k - 4)
            if valid(k - 1):
                a2_stats(k - 1)
            if valid(k - 2):
                a2b_rs(k - 2)
            if valid(k - 5):
                b_proj(k - 5)
            if valid(k - 6):
                c_scores(k - 6)
            if valid(k - 7):
                d_y(k - 7)
            if valid(k - 8):
                dt_out(k - 8)


def _install_ntff_hook():
    """Register the axon NTFF profiling hook (the image's antenv lacks
    axon_hooks, so boot skipped registration). Trace-only; best-effort."""
    try:
        import types

        import antenv

        if getattr(antenv, "axon_hooks", None) is not None:
            return
        mod = types.ModuleType("antenv.axon_hooks")
        _h = [None]
        mod.set_axon_ntff_profile_hook = lambda h: _h.__setitem__(0, h)
        mod.get_axon_ntff_profile_hook = lambda: _h[0]
        sys.modules["antenv.axon_hooks"] = mod
        antenv.axon_hooks = mod
        from trn_agent_boot.trn_boot import _ntff_profile_via_ctypes

        hook = _ntff_profile_via_ctypes("/opt/axon/libaxon_pjrt.so")
        if hook is not None:
            mod.set_axon_ntff_profile_hook(hook)
    except Exception as e:  # noqa: BLE001
        print(f"ntff hook install failed (timing unavailable): {e}")


def kernel(x, ln_gamma, ln_beta, W_qkv):
    x = np.asarray(x, dtype=np.float32)
    ln_gamma = np.asarray(ln_gamma, dtype=np.float32)
    ln_beta = np.asarray(ln_beta, dtype=np.float32)
    W_qkv = np.asarray(W_qkv, dtype=np.float32)
    assert x.shape == (B, H, W, C) and W_qkv.shape == (C, F2)

    # fold gamma/beta into the projection (1x1 conv has no bias of its own)
    Wp = (ln_gamma.astype(np.float64)[:, None]
          * W_qkv.astype(np.float64)).astype(np.float32)
    bW = (ln_beta.astype(np.float64)
          @ W_qkv.astype(np.float64)).astype(np.float32)
    with_bias = bool(np.any(bW != 0.0))

    key = with_bias
    if key not in _NC_CACHE:
        _NC_CACHE[key] = _build(with_bias)
    nc = _NC_CACHE[key]

    in_maps = []
    for b in range(B):
        m = {
            "x": np.ascontiguousarray(x[b]),
            "wqk": np.ascontiguousarray(Wp[:, :256].reshape(2, 128, 256)),
            "wv": np.ascontiguousarray(Wp[:, 256:].reshape(2, 128, 256)),
        }
        if with_bias:
            m["bqk"] = np.ascontiguousarray(bW[:256].reshape(2, 128))
            m["bv"] = np.ascontiguousarray(bW[256:])
        in_maps.append(m)

    trace = os.environ.get("KERNEL_TRACE", "") == "1"
    if trace:
        _install_ntff_hook()
    res = run_bass_kernel_spmd(nc, in_maps, core_ids=list(range(B)), trace=trace)
    if trace and res.exec_time_ns is not None:
        print(f"HW exec time: {res.exec_time_ns} ns")
        if res.instructions_and_trace is not None:
            print(f"trace: {res.instructions_and_trace[1]}")
    out = np.stack([res.results[b]["out"] for b in range(B)], axis=0)
    return out.reshape(B, H, W, C).astype(np.float32, copy=False)
